# revision 8
# baseline (speedup 1.0000x reference)
"""DLinear Trainium2 kernel (nn_DLinear_45990509805636).

Math: with T=17 and KERNEL_SIZE=37 (PAD=18), every moving-average window
covers the whole sequence plus replicated edges, so

    trend[b,t,:] = (S + (18-t)*x0 + (t+2)*x16) / 37,   S = sum_t x[:,t,:]
    out = seasonal @ Ws[t] + trend @ Wt[t] + (bs+bt)[t]
        = x_t @ Ws[t] + trend_raw_t @ Wd[t] + bias[t],
    Wd = (Wt - Ws)/37 (host-folded), trend_raw_t = P + t*Q,
    P = S + 18*x0 + 2*x16, Q = x16 - x0.

Device per core (batch shard of 512 rows), all bf16 matmuls:
  - x.T resident in SBUF as [c%128, t, c//128, b]; S chained on DVE behind
    the 2-token x transfers; the post-x tail (S+=x16, P) runs per-128-column
    chunk so phase-B can start ~1.5us after the last x byte lands; trend(0)
    aliases P (no copy), Q overwrites dead S.
  - prologue (t < PRE): phase-split. x@Ws groups (4 MMs) fill the PE while
    x streams at priority; ACT copies psum->outa; after trend is ready the
    trend@Wd group is joined with outa by one DVE STT.
  - steady state (t >= PRE): ONE psum group of 8 MMs (4 x@Ws + 4 trend@Wd),
    ACT copies psum->f16, paired 2-token stores dispatched from ACT.
  - bias is NOT added on device: the host epilogue adds (bs+bt) during the
    f16->f32 upcast (saves 68 K=1 broadcast matmuls ~27us of PE time).
  - a burst of zero matmuls at t=0 warms the PE HAM clock-gate during the
    initial DMA wait so the first real matmuls run at 2.4GHz.

Sharding: data-parallel over batch, 8 cores x 512 rows; weights replicated.
"""

import sys

sys.path.insert(0, "/opt/trn_rl_repo")

import numpy as np
import ml_dtypes

from concourse import bacc
import concourse.mybir as mybir
import concourse.tile as tile
from concourse.bass_utils import run_bass_kernel_spmd

dt = mybir.dt

B, T, C, D = 4096, 17, 512, 512
NCORES = 8
BC = B // NCORES          # 512 batch rows per core
KC = C // 128             # 4 contraction chunks
JB = BC // 128            # 4 output-row tiles per core

PRE = 9                   # tokens handled phase-split to fill the DMA prologue
WARM = 10                 # zero-matmuls to warm the HAM clock gate


def build():
    idt = dt.bfloat16
    nc = bacc.Bacc(None, target_bir_lowering=False, name="dlinear_v3")
    xt = nc.dram_tensor("xt", [T, 128, KC, BC], idt, kind="ExternalInput")
    wst = nc.dram_tensor("wst", [T, 128, KC, D], idt, kind="ExternalInput")
    wdt = nc.dram_tensor("wdt", [T, 128, KC, D], idt, kind="ExternalInput")
    out = nc.dram_tensor("out", [BC, T, D], dt.float16, kind="ExternalOutput")

    with tile.TileContext(nc) as tc:
        with (
            tc.tile_pool(name="xres", bufs=1) as xres,
            tc.tile_pool(name="stats", bufs=1) as stats,
            tc.tile_pool(name="wsbuf", bufs=PRE + 1) as wsbuf,
            tc.tile_pool(name="wdbuf", bufs=5) as wdbuf,
            tc.tile_pool(name="tbuf", bufs=2) as tbuf,
            tc.tile_pool(name="abuf", bufs=4 * PRE + 2) as abuf,
            tc.tile_pool(name="obuf", bufs=8) as obuf,
            tc.tile_pool(name="psum", bufs=8, space="PSUM") as psum,
        ):
            xsb = xres.tile([128, T, KC, BC], idt)

            # HAM warm-up: matmuls on zeros while the first DMAs are in flight
            zs = stats.tile([128, D], idt)
            nc.vector.memset(zs, 0.0)
            wps = psum.tile([128, D], dt.float32, tag="ps", name="warm")
            for i in range(WARM):
                nc.tensor.matmul(wps, zs[:, 0:128], zs,
                                 start=(i == 0), stop=(i == WARM - 1))

            # ---- DMA schedule (SP): x gets priority; ws paced between x
            # pairs; wd + late ws strictly after x so trend unblocks early.
            ws_tiles = {}

            def load_ws(t, split=False):
                w = wsbuf.tile([128, KC, D], idt, tag="ws", name="ws")
                if split:
                    nc.sync.dma_start(w[:, 0:2], wst[t, :, 0:2])
                    nc.sync.dma_start(w[:, 2:4], wst[t, :, 2:4])
                else:
                    nc.sync.dma_start(w, wst[t])
                ws_tiles[t] = w

            def load_x(t0, t1):
                nc.sync.dma_start(
                    xsb[:, t0:t1],
                    xt[t0:t1].rearrange("t p k b -> p t k b"),
                )

            load_ws(0, split=True)
            load_x(0, 1)
            load_ws(1)
            load_x(1, 3)
            load_x(3, 5)
            load_ws(2)
            load_x(5, 7)
            load_x(7, 9)
            load_ws(3)
            load_x(9, 11)
            load_x(11, 13)
            load_ws(4)
            load_x(13, 15)
            load_x(15, 17)
            for t in range(5, PRE):
                load_ws(t)

            wd_tiles = {}

            def load_wd(t):
                w = wdbuf.tile([128, KC, D], idt, tag="wd", name="wd")
                nc.sync.dma_start(w, wdt[t])
                wd_tiles[t] = w

            load_wd(0)
            load_wd(1)

            # ---- phase-A prologue: pure x@Ws groups, ACT drains psum->outa
            def emit_phase_a(t, j):
                psa = psum.tile([128, D], dt.float32, tag="ps", name="psa")
                for k in range(KC):
                    nc.tensor.matmul(
                        psa, xsb[:, t, k, j * 128:(j + 1) * 128],
                        ws_tiles[t][:, k],
                        start=(k == 0), stop=(k == KC - 1),
                    )
                outa = abuf.tile([128, D], idt, tag="outa", name="outa")
                nc.scalar.copy(outa, psa)
                return outa

            outa_pre = {}
            for t in range(PRE):
                for j in range(JB):
                    outa_pre[(t, j)] = emit_phase_a(t, j)

            # ---- S chained full-width behind x arrivals; tail per-chunk so
            # phase-B(0) starts right after the last x lands.
            S = stats.tile([128, KC, BC], idt)
            P = stats.tile([128, KC, BC], idt)
            tmp = stats.tile([128, KC, BC], idt)
            nc.vector.tensor_tensor(S[:], xsb[:, 0], xsb[:, 1], mybir.AluOpType.add)
            for t in range(2, 16):
                nc.vector.tensor_tensor(S[:], S[:], xsb[:, t], mybir.AluOpType.add)
            for k in range(KC):
                nc.vector.tensor_tensor(S[:, k], S[:, k], xsb[:, 16, k],
                                        mybir.AluOpType.add)
                nc.vector.tensor_scalar_mul(tmp[:, k], xsb[:, 0, k], 18.0)
                nc.vector.tensor_tensor(P[:, k], tmp[:, k], S[:, k],
                                        mybir.AluOpType.add)
                nc.vector.tensor_scalar_mul(tmp[:, k], xsb[:, 16, k], 2.0)
                nc.vector.tensor_tensor(P[:, k], tmp[:, k], P[:, k],
                                        mybir.AluOpType.add)
            # Q overwrites dead S; per-chunk so trend(1) can chase phase-B(0)
            Q = S
            for k in range(KC):
                nc.vector.tensor_tensor(Q[:, k], xsb[:, 16, k], xsb[:, 0, k],
                                        mybir.AluOpType.subtract)

            def make_trend(t):
                if t == 0:
                    return P
                trend = tbuf.tile([128, KC, BC], idt, tag="trend", name="trend")
                for k in range(KC):
                    nc.vector.tensor_scalar_mul(trend[:, k], Q[:, k], float(t))
                    nc.vector.tensor_tensor(trend[:, k], trend[:, k], P[:, k],
                                            mybir.AluOpType.add)
                return trend

            osb_cur = {}

            def store_out(t, j, osb):
                if t % 2 == 1:
                    nc.scalar.dma_start(
                        out[j * 128:(j + 1) * 128, t - 1:t + 1, :], osb)
                elif t == T - 1:
                    nc.scalar.dma_start(
                        out[j * 128:(j + 1) * 128, t:t + 1, :], osb[:, 0:1])

            def get_osb(t, j):
                if t % 2 == 0:
                    osb = obuf.tile([128, 2, D], dt.float16, tag="osb", name="osb")
                    osb_cur[j] = osb
                    return osb, osb[:, 0]
                return osb_cur[j], osb_cur[j][:, 1]

            # ---- phase-B for prologue tokens: 4-MM trend groups + DVE join
            for t in range(PRE):
                if t >= 2:
                    load_wd(t)
                trend = make_trend(t)
                for j in range(JB):
                    psb = psum.tile([128, D], dt.float32, tag="ps", name="psb")
                    for k in range(KC):
                        nc.tensor.matmul(
                            psb, trend[:, k, j * 128:(j + 1) * 128],
                            wd_tiles[t][:, k],
                            start=(k == 0), stop=(k == KC - 1),
                        )
                    osb, slot = get_osb(t, j)
                    nc.vector.scalar_tensor_tensor(
                        slot, psb, 1.0, outa_pre.pop((t, j)),
                        mybir.AluOpType.mult, mybir.AluOpType.add,
                    )
                    store_out(t, j, osb)

            # prime steady-state loads (consumption order, 2-token lookahead)
            for t in range(PRE, min(PRE + 2, T)):
                load_ws(t)
                load_wd(t)

            # ---- steady state: one 8-MM group per (t, j), ACT drains to f16
            for t in range(PRE, T):
                if t + 2 < T:
                    load_ws(t + 2)
                    load_wd(t + 2)
                trend = make_trend(t)
                for j in range(JB):
                    ps = psum.tile([128, D], dt.float32, tag="ps", name="ps")
                    for k in range(KC):
                        nc.tensor.matmul(
                            ps, xsb[:, t, k, j * 128:(j + 1) * 128],
                            ws_tiles[t][:, k],
                            start=(k == 0), stop=False,
                        )
                    for k in range(KC):
                        nc.tensor.matmul(
                            ps, trend[:, k, j * 128:(j + 1) * 128],
                            wd_tiles[t][:, k],
                            start=False, stop=(k == KC - 1),
                        )
                    osb, slot = get_osb(t, j)
                    nc.scalar.copy(slot, ps)
                    store_out(t, j, osb)
    nc.compile()
    return nc


_NC_CACHE = {}


def _get_nc(mode="bf16"):
    if "nc" not in _NC_CACHE:
        _NC_CACHE["nc"] = build()
    return _NC_CACHE["nc"]


MODE = "bf16"


def kernel(x, W_seasonal, b_seasonal, W_trend, b_trend, _trace=False):
    npdt = ml_dtypes.bfloat16
    nc = _get_nc()

    def to_tpkd(w):  # [T, D, C] -> [T, 128, KC, D] (c-major on partitions)
        wt = w.transpose(0, 2, 1).reshape(T, KC, 128, D)
        return np.ascontiguousarray(wt.transpose(0, 2, 1, 3))

    wst = to_tpkd(W_seasonal).astype(npdt)
    wdt = to_tpkd((W_trend - W_seasonal) / 37.0).astype(npdt)
    bias = (b_seasonal + b_trend).astype(np.float32)  # host epilogue

    in_maps = []
    for i in range(NCORES):
        xs = x[i * BC:(i + 1) * BC]                    # [BC, T, C]
        xti = xs.transpose(1, 2, 0).reshape(T, KC, 128, BC)
        xti = np.ascontiguousarray(xti.transpose(0, 2, 1, 3)).astype(npdt)
        in_maps.append({"xt": xti, "wst": wst, "wdt": wdt})

    res = run_bass_kernel_spmd(
        nc, in_maps, core_ids=list(range(NCORES)), trace=_trace
    )
    outp = np.concatenate([r["out"] for r in res.results], axis=0)
    outp = outp.astype(np.float32)
    outp += bias[None]
    if _trace:
        return outp, res
    return outp


if __name__ == "__main__":
    rng = np.random.default_rng(0)
    x = rng.standard_normal((B, T, C), dtype=np.float32)
    Ws = rng.uniform(-0.04, 0.04, (T, D, C)).astype(np.float32)
    Wt = rng.uniform(-0.04, 0.04, (T, D, C)).astype(np.float32)
    bs = rng.uniform(-0.04, 0.04, (T, D)).astype(np.float32)
    bt = rng.uniform(-0.04, 0.04, (T, D)).astype(np.float32)
    o = kernel(x, Ws, bs, Wt, bt)
    print("out shape:", o.shape, o.dtype)


# revision 12
# speedup vs baseline: 1.0199x; 1.0199x over previous
"""DLinear Trainium2 kernel (nn_DLinear_45990509805636).

Math: with T=17 and KERNEL_SIZE=37 (PAD=18), every moving-average window
covers the whole sequence plus replicated edges, so

    trend[b,t,:] = (S + (18-t)*x0 + (t+2)*x16) / 37,   S = sum_t x[:,t,:]
    out = seasonal @ Ws[t] + trend @ Wt[t] + (bs+bt)[t]
        = x_t @ Ws[t] + trend_raw_t @ Wd[t] + bias[t],
    Wd = (Wt - Ws)/37 (host-folded), trend_raw_t = P + t*Q,
    P = S + 18*x0 + 2*x16, Q = x16 - x0.

Device per core (batch shard of 512 rows), all bf16 matmuls, PE-bound at
544 N=512 matmuls (~124us); the schedule aims to keep the PE gapless:
  - x.T resident in SBUF as [c%128, t, c//128, b]; S chained on DVE behind
    the 2-token x transfers; the post-x tail (S+=x16, P) runs per-128-column
    chunk so trend unblocks ~1.5us after the last x byte lands; trend(0)
    aliases P (no copy), Q overwrites dead S.
  - prologue (t < PRE=12): phase-split. x@Ws groups (4 MMs) fill the PE
    while x/ws stream in (ws interleaved just-in-time); ACT drains psum
    straight into the f16 output pair-tile; the trend@Wd group later joins
    IN PLACE via one DVE STT (no separate outa buffers).
  - post-prologue: phase-B tokens interleaved with steady single-group
    tokens (8 MMs, ACT drain) so the DVE combine load stays under the PE
    pace; paired 2-token stores dispatched from ACT.
  - bias is NOT added on device: the host epilogue adds (bs+bt) during the
    f16->f32 upcast (saves 68 K=1 broadcast matmuls ~27us of PE time).
  - a burst of zero matmuls at t=0 warms the PE HAM clock-gate during the
    initial DMA wait so the first real matmuls run at 2.4GHz.

Sharding: data-parallel over batch, 8 cores x 512 rows; weights replicated.
"""

import sys

sys.path.insert(0, "/opt/trn_rl_repo")

import numpy as np
import ml_dtypes

from concourse import bacc
import concourse.mybir as mybir
import concourse.tile as tile
from concourse.bass_utils import run_bass_kernel_spmd

dt = mybir.dt

B, T, C, D = 4096, 17, 512, 512
NCORES = 8
BC = B // NCORES          # 512 batch rows per core
KC = C // 128             # 4 contraction chunks
JB = BC // 128            # 4 output-row tiles per core

PRE = 12                  # tokens handled phase-split (must be even)
WARM = 10                 # zero-matmuls to warm the HAM clock gate


def build():
    idt = dt.bfloat16
    nc = bacc.Bacc(None, target_bir_lowering=False, name="dlinear_v4")
    xt = nc.dram_tensor("xt", [T, 128, KC, BC], idt, kind="ExternalInput")
    wst = nc.dram_tensor("wst", [T, 128, KC, D], idt, kind="ExternalInput")
    wdt = nc.dram_tensor("wdt", [T, 128, KC, D], idt, kind="ExternalInput")
    out = nc.dram_tensor("out", [BC, T, D], dt.float16, kind="ExternalOutput")

    with tile.TileContext(nc) as tc:
        with (
            tc.tile_pool(name="xres", bufs=1) as xres,
            tc.tile_pool(name="stats", bufs=1) as stats,
            tc.tile_pool(name="wsbuf", bufs=10) as wsbuf,
            tc.tile_pool(name="wdbuf", bufs=6) as wdbuf,
            tc.tile_pool(name="tbuf", bufs=2) as tbuf,
            tc.tile_pool(name="obuf", bufs=28) as obuf,
            tc.tile_pool(name="psum", bufs=8, space="PSUM") as psum,
        ):
            xsb = xres.tile([128, T, KC, BC], idt)
            S = stats.tile([128, KC, BC], idt)
            P = stats.tile([128, KC, BC], idt)

            # HAM warm-up: matmuls on zeros while the first DMAs are in
            # flight (P's storage is scratch until the real P is written)
            nc.vector.memset(P[:, 0], 0.0)
            wps = psum.tile([128, D], dt.float32, tag="ps", name="warm")
            for i in range(WARM):
                nc.tensor.matmul(wps, P[:, 0, 0:128], P[:, 0],
                                 start=(i == 0), stop=(i == WARM - 1))

            # ---- DMA schedule (SP): ws[t] just-in-time between x pairs so
            # the statically-ordered phase-A stream never head-of-line blocks
            ws_tiles = {}

            def load_ws(t, split=False):
                w = wsbuf.tile([128, KC, D], idt, tag="ws", name="ws")
                if split:
                    nc.sync.dma_start(w[:, 0:2], wst[t, :, 0:2])
                    nc.sync.dma_start(w[:, 2:4], wst[t, :, 2:4])
                else:
                    nc.sync.dma_start(w, wst[t])
                ws_tiles[t] = w

            def load_x(t0, t1):
                nc.sync.dma_start(
                    xsb[:, t0:t1],
                    xt[t0:t1].rearrange("t p k b -> p t k b"),
                )

            load_ws(0, split=True)
            load_x(0, 1)
            load_ws(1)
            load_x(1, 3)
            load_ws(2)
            load_x(3, 5)
            load_ws(3)
            load_x(5, 7)
            load_ws(4)
            load_x(7, 9)
            load_ws(5)
            load_x(9, 11)
            load_ws(6)
            load_x(11, 13)
            load_ws(7)
            load_x(13, 15)
            load_ws(8)
            load_x(15, 17)
            for t in range(9, PRE):
                load_ws(t)

            wd_tiles = {}

            def load_wd(t):
                w = wdbuf.tile([128, KC, D], idt, tag="wd", name="wd")
                nc.sync.dma_start(w, wdt[t])
                wd_tiles[t] = w

            for t in range(5):
                load_wd(t)

            # ---- f16 output pair-tiles [128, 2, D]: slot t%2
            osb_pairs = {}

            def pair_slot(t, j):
                key = (t // 2, j)
                if key not in osb_pairs:
                    osb_pairs[key] = obuf.tile(
                        [128, 2, D], dt.float16, tag="osb", name="osb")
                return osb_pairs[key], osb_pairs[key][:, t % 2]

            def store_out(t, j):
                osb = osb_pairs[(t // 2, j)]
                if t % 2 == 1:
                    nc.scalar.dma_start(
                        out[j * 128:(j + 1) * 128, t - 1:t + 1, :], osb)
                elif t == T - 1:
                    nc.scalar.dma_start(
                        out[j * 128:(j + 1) * 128, t:t + 1, :], osb[:, 0:1])

            # ---- phase-A prologue: x@Ws -> psum -> ACT drain to f16 slot
            for t in range(PRE):
                for j in range(JB):
                    psa = psum.tile([128, D], dt.float32, tag="ps", name="psa")
                    for k in range(KC):
                        nc.tensor.matmul(
                            psa, xsb[:, t, k, j * 128:(j + 1) * 128],
                            ws_tiles[t][:, k],
                            start=(k == 0), stop=(k == KC - 1),
                        )
                    _, slot = pair_slot(t, j)
                    nc.scalar.copy(slot, psa)

            # ---- S chained full-width behind x arrivals; tail per-chunk
            nc.vector.tensor_tensor(S[:], xsb[:, 0], xsb[:, 1], mybir.AluOpType.add)
            for t in range(2, 16):
                nc.vector.tensor_tensor(S[:], S[:], xsb[:, t], mybir.AluOpType.add)
            for k in range(KC):
                nc.vector.tensor_tensor(S[:, k], S[:, k], xsb[:, 16, k],
                                        mybir.AluOpType.add)
                nc.vector.scalar_tensor_tensor(
                    P[:, k], xsb[:, 0, k], 18.0, S[:, k],
                    mybir.AluOpType.mult, mybir.AluOpType.add)
                nc.vector.scalar_tensor_tensor(
                    P[:, k], xsb[:, 16, k], 2.0, P[:, k],
                    mybir.AluOpType.mult, mybir.AluOpType.add)
            # Q overwrites dead S; per-chunk so trend(1) can chase phase-B(0)
            Q = S
            for k in range(KC):
                nc.vector.tensor_tensor(Q[:, k], xsb[:, 16, k], xsb[:, 0, k],
                                        mybir.AluOpType.subtract)

            def make_trend(t):
                if t == 0:
                    return P
                trend = tbuf.tile([128, KC, BC], idt, tag="trend", name="trend")
                nc.vector.tensor_scalar_mul(trend[:], Q[:], float(t))
                nc.vector.tensor_tensor(trend[:], trend[:], P[:],
                                        mybir.AluOpType.add)
                return trend

            def emit_phase_b(t):
                # trend@Wd joins the stored x@Ws part in place (DVE STT)
                trend = make_trend(t)
                for j in range(JB):
                    psb = psum.tile([128, D], dt.float32, tag="ps", name="psb")
                    for k in range(KC):
                        nc.tensor.matmul(
                            psb, trend[:, k, j * 128:(j + 1) * 128],
                            wd_tiles[t][:, k],
                            start=(k == 0), stop=(k == KC - 1),
                        )
                    _, slot = pair_slot(t, j)
                    nc.vector.scalar_tensor_tensor(
                        slot, psb, 1.0, slot,
                        mybir.AluOpType.mult, mybir.AluOpType.add,
                    )
                    store_out(t, j)

            def emit_steady(t):
                # one 8-MM group per (t, j), ACT drains straight to f16
                trend = make_trend(t)
                for j in range(JB):
                    ps = psum.tile([128, D], dt.float32, tag="ps", name="ps")
                    for k in range(KC):
                        nc.tensor.matmul(
                            ps, xsb[:, t, k, j * 128:(j + 1) * 128],
                            ws_tiles[t][:, k],
                            start=(k == 0), stop=False,
                        )
                    for k in range(KC):
                        nc.tensor.matmul(
                            ps, trend[:, k, j * 128:(j + 1) * 128],
                            wd_tiles[t][:, k],
                            start=False, stop=(k == KC - 1),
                        )
                    _, slot = pair_slot(t, j)
                    nc.scalar.copy(slot, ps)
                    store_out(t, j)

            # ---- post-prologue: interleave phase-B with steady tokens so
            # the DVE combine load stays below the PE pace
            sched = []
            steady_iter = list(range(PRE, T))
            for i, t in enumerate(range(PRE)):
                sched.append(("B", t))
                if i < len(steady_iter):
                    sched.append(("s", steady_iter[i]))
            loads_order = [t for _, t in sched]

            li = 0  # rolling load emission: wd for B tokens, ws+wd for steady
            def emit_loads_until(n):
                nonlocal li
                while li < n:
                    kind, t = sched[li]
                    if kind == "B":
                        if t >= 5:
                            load_wd(t)
                    else:
                        load_ws(t)
                        load_wd(t)
                    li += 1

            emit_loads_until(4)
            for i, (kind, t) in enumerate(sched):
                emit_loads_until(min(i + 4, len(sched)))
                if kind == "B":
                    emit_phase_b(t)
                else:
                    emit_steady(t)
    nc.compile()
    return nc


_NC_CACHE = {}


def _get_nc(mode="bf16"):
    if "nc" not in _NC_CACHE:
        _NC_CACHE["nc"] = build()
    return _NC_CACHE["nc"]


MODE = "bf16"


def kernel(x, W_seasonal, b_seasonal, W_trend, b_trend, _trace=False):
    npdt = ml_dtypes.bfloat16
    nc = _get_nc()

    def to_tpkd(w):  # [T, D, C] -> [T, 128, KC, D] (c-major on partitions)
        wt = w.transpose(0, 2, 1).reshape(T, KC, 128, D)
        return np.ascontiguousarray(wt.transpose(0, 2, 1, 3))

    wst = to_tpkd(W_seasonal).astype(npdt)
    wdt = to_tpkd((W_trend - W_seasonal) / 37.0).astype(npdt)
    bias = (b_seasonal + b_trend).astype(np.float32)  # host epilogue

    in_maps = []
    for i in range(NCORES):
        xs = x[i * BC:(i + 1) * BC]                    # [BC, T, C]
        xti = xs.transpose(1, 2, 0).reshape(T, KC, 128, BC)
        xti = np.ascontiguousarray(xti.transpose(0, 2, 1, 3)).astype(npdt)
        in_maps.append({"xt": xti, "wst": wst, "wdt": wdt})

    res = run_bass_kernel_spmd(
        nc, in_maps, core_ids=list(range(NCORES)), trace=_trace
    )
    outp = np.concatenate([r["out"] for r in res.results], axis=0)
    outp = outp.astype(np.float32)
    outp += bias[None]
    if _trace:
        return outp, res
    return outp


if __name__ == "__main__":
    rng = np.random.default_rng(0)
    x = rng.standard_normal((B, T, C), dtype=np.float32)
    Ws = rng.uniform(-0.04, 0.04, (T, D, C)).astype(np.float32)
    Wt = rng.uniform(-0.04, 0.04, (T, D, C)).astype(np.float32)
    bs = rng.uniform(-0.04, 0.04, (T, D)).astype(np.float32)
    bt = rng.uniform(-0.04, 0.04, (T, D)).astype(np.float32)
    o = kernel(x, Ws, bs, Wt, bt)
    print("out shape:", o.shape, o.dtype)


# revision 16
# speedup vs baseline: 1.0346x; 1.0145x over previous
"""DLinear Trainium2 kernel (nn_DLinear_45990509805636).

Math: with T=17 and KERNEL_SIZE=37 (PAD=18), every moving-average window
covers the whole sequence plus replicated edges, so

    trend[b,t,:] = (S + (18-t)*x0 + (t+2)*x16) / 37,   S = sum_t x[:,t,:]
    out = seasonal @ Ws[t] + trend @ Wt[t] + (bs+bt)[t]
        = x_t @ Ws[t] + trend_raw_t @ Wd[t] + bias[t],
    Wd = (Wt - Ws)/37 (host-folded), trend_raw_t = P + t*Q,
    P = S + 18*x0 + 2*x16, Q = x16 - x0.

Device per core (batch shard of 512 rows), all bf16 matmuls, PE-bound at
544 N=512 matmuls (~124us); the schedule aims to keep the PE gapless:
  - x.T resident in SBUF as [c%128, t, c//128, b]; S chained on DVE behind
    the 2-token x transfers; the post-x tail (S+=x16, P) runs per-128-column
    chunk so trend unblocks ~1.5us after the last x byte lands; trend(0)
    aliases P (no copy), Q overwrites dead S.
  - prologue (t < PRE=12): phase-split. x@Ws groups (4 MMs) fill the PE
    while x/ws stream in (ws interleaved just-in-time); ACT drains psum
    straight into the f16 output pair-tile; the trend@Wd group later joins
    IN PLACE via one DVE STT (no separate outa buffers).
  - post-prologue: phase-B tokens interleaved with steady single-group
    tokens (8 MMs, ACT drain) so the DVE combine load stays under the PE
    pace; paired 2-token stores dispatched from ACT.
  - bias is NOT added on device: the host epilogue adds (bs+bt) during the
    f16->f32 upcast (saves 68 K=1 broadcast matmuls ~27us of PE time).
  - a burst of zero matmuls at t=0 warms the PE HAM clock-gate during the
    initial DMA wait so the first real matmuls run at 2.4GHz.

Sharding: data-parallel over batch, 8 cores x 512 rows; weights replicated.
"""

import sys

sys.path.insert(0, "/opt/trn_rl_repo")

import numpy as np
import ml_dtypes

from concourse import bacc
import concourse.mybir as mybir
import concourse.tile as tile
from concourse.bass_utils import run_bass_kernel_spmd

dt = mybir.dt

B, T, C, D = 4096, 17, 512, 512
NCORES = 8
BC = B // NCORES          # 512 batch rows per core
KC = C // 128             # 4 contraction chunks
JB = BC // 128            # 4 output-row tiles per core

PRE = 12                  # tokens handled phase-split (must be even)
WARM = 10                 # zero-matmuls to warm the HAM clock gate


def build():
    idt = dt.bfloat16
    nc = bacc.Bacc(None, target_bir_lowering=False, name="dlinear_v4")
    xt = nc.dram_tensor("xt", [T, 128, KC, BC], idt, kind="ExternalInput")
    wst = nc.dram_tensor("wst", [T, 128, KC, D], idt, kind="ExternalInput")
    wdt = nc.dram_tensor("wdt", [T, 128, KC, D], idt, kind="ExternalInput")
    out = nc.dram_tensor("out", [BC, T, D], dt.float16, kind="ExternalOutput")

    with tile.TileContext(nc) as tc:
        with (
            tc.tile_pool(name="xres", bufs=1) as xres,
            tc.tile_pool(name="stats", bufs=1) as stats,
            tc.tile_pool(name="wsbuf", bufs=10) as wsbuf,
            tc.tile_pool(name="wdbuf", bufs=7) as wdbuf,
            tc.tile_pool(name="tbuf", bufs=2) as tbuf,
            tc.tile_pool(name="obuf", bufs=52) as obuf,
            tc.tile_pool(name="psum", bufs=8, space="PSUM") as psum,
        ):
            xsb = xres.tile([128, T, KC, BC], idt)
            S = stats.tile([128, KC, BC], idt)
            P = stats.tile([128, KC, BC], idt)

            # HAM warm-up: matmuls on zeros while the first DMAs are in
            # flight (P's storage is scratch until the real P is written).
            # A second burst gates on the ws0 DMA so the PE-idle window
            # before the first real matmul stays under the ~3.4us HAM
            # re-throttle threshold.
            nc.vector.memset(P[:, 0], 0.0)
            wps = psum.tile([128, D], dt.float32, tag="ps", name="warm")
            for i in range(WARM):
                nc.tensor.matmul(wps, P[:, 0, 0:128], P[:, 0],
                                 start=(i == 0), stop=(i == WARM - 1))

            # ---- DMA schedule (SP): ws[t] just-in-time between x pairs so
            # the statically-ordered phase-A stream never head-of-line blocks
            ws_tiles = {}

            def load_ws(t, split=False):
                w = wsbuf.tile([128, KC, D], idt, tag="ws", name="ws")
                if split:
                    nc.sync.dma_start(w[:, 0:2], wst[t, :, 0:2])
                    nc.sync.dma_start(w[:, 2:4], wst[t, :, 2:4])
                else:
                    nc.sync.dma_start(w, wst[t])
                ws_tiles[t] = w

            def load_x(t0, t1):
                nc.sync.dma_start(
                    xsb[:, t0:t1],
                    xt[t0:t1].rearrange("t p k b -> p t k b"),
                )

            load_ws(0, split=True)
            load_x(0, 1)
            load_ws(1)
            load_x(1, 3)
            load_ws(2)
            load_x(3, 5)
            load_ws(3)
            load_x(5, 7)
            load_ws(4)
            load_x(7, 9)
            load_ws(5)
            load_x(9, 11)
            load_ws(6)
            load_x(11, 13)
            load_ws(7)
            load_x(13, 15)
            load_ws(8)
            load_x(15, 17)
            for t in range(9, PRE):
                load_ws(t)

            # warm-up burst 2: moving operand is the (garbage-ok) ws0 tile,
            # so these run right after the ws0 DMA lands
            wps2 = psum.tile([128, D], dt.float32, tag="ps", name="warm2")
            for i in range(4):
                nc.tensor.matmul(wps2, P[:, 0, 0:128], ws_tiles[0][:, 0],
                                 start=(i == 0), stop=(i == 3))

            wd_tiles = {}

            def load_wd(t):
                w = wdbuf.tile([128, KC, D], idt, tag="wd", name="wd")
                nc.sync.dma_start(w, wdt[t])
                wd_tiles[t] = w

            for t in range(5):
                load_wd(t)

            # ---- f16 per-token output tiles
            osb_tiles = {}

            def pair_slot(t, j):
                key = (t, j)
                if key not in osb_tiles:
                    osb_tiles[key] = obuf.tile(
                        [128, 1, D], dt.float16, tag="osb", name="osb")
                return osb_tiles[key], osb_tiles[key][:, 0]

            def store_out(t, j):
                nc.scalar.dma_start(
                    out[j * 128:(j + 1) * 128, t:t + 1, :],
                    osb_tiles.pop((t, j)))

            # ---- phase-A prologue: x@Ws -> psum -> ACT drain to f16 slot
            for t in range(PRE):
                for j in range(JB):
                    psa = psum.tile([128, D], dt.float32, tag="ps", name="psa")
                    for k in range(KC):
                        nc.tensor.matmul(
                            psa, xsb[:, t, k, j * 128:(j + 1) * 128],
                            ws_tiles[t][:, k],
                            start=(k == 0), stop=(k == KC - 1),
                        )
                    _, slot = pair_slot(t, j)
                    nc.scalar.copy(slot, psa)

            # ---- S chained full-width behind x arrivals; tail per-chunk
            nc.vector.tensor_tensor(S[:], xsb[:, 0], xsb[:, 1], mybir.AluOpType.add)
            for t in range(2, 16):
                nc.vector.tensor_tensor(S[:], S[:], xsb[:, t], mybir.AluOpType.add)
            for k in range(KC):
                nc.vector.tensor_tensor(S[:, k], S[:, k], xsb[:, 16, k],
                                        mybir.AluOpType.add)
                nc.vector.scalar_tensor_tensor(
                    P[:, k], xsb[:, 0, k], 18.0, S[:, k],
                    mybir.AluOpType.mult, mybir.AluOpType.add)
                nc.vector.scalar_tensor_tensor(
                    P[:, k], xsb[:, 16, k], 2.0, P[:, k],
                    mybir.AluOpType.mult, mybir.AluOpType.add)
            # Q overwrites dead S; per-chunk so trend(1) can chase phase-B(0)
            Q = S
            for k in range(KC):
                nc.vector.tensor_tensor(Q[:, k], xsb[:, 16, k], xsb[:, 0, k],
                                        mybir.AluOpType.subtract)

            def make_trend(t):
                if t == 0:
                    return P
                trend = tbuf.tile([128, KC, BC], idt, tag="trend", name="trend")
                nc.vector.tensor_scalar_mul(trend[:], Q[:], float(t))
                nc.vector.tensor_tensor(trend[:], trend[:], P[:],
                                        mybir.AluOpType.add)
                return trend

            def emit_phase_b(t):
                # trend@Wd joins the stored x@Ws part in place (DVE STT)
                trend = make_trend(t)
                for j in range(JB):
                    psb = psum.tile([128, D], dt.float32, tag="ps", name="psb")
                    for k in range(KC):
                        nc.tensor.matmul(
                            psb, trend[:, k, j * 128:(j + 1) * 128],
                            wd_tiles[t][:, k],
                            start=(k == 0), stop=(k == KC - 1),
                        )
                    _, slot = pair_slot(t, j)
                    nc.vector.scalar_tensor_tensor(
                        slot, psb, 1.0, slot,
                        mybir.AluOpType.mult, mybir.AluOpType.add,
                    )
                    store_out(t, j)

            def emit_steady(t):
                # one 8-MM group per (t, j), ACT drains straight to f16
                trend = make_trend(t)
                for j in range(JB):
                    ps = psum.tile([128, D], dt.float32, tag="ps", name="ps")
                    for k in range(KC):
                        nc.tensor.matmul(
                            ps, xsb[:, t, k, j * 128:(j + 1) * 128],
                            ws_tiles[t][:, k],
                            start=(k == 0), stop=False,
                        )
                    for k in range(KC):
                        nc.tensor.matmul(
                            ps, trend[:, k, j * 128:(j + 1) * 128],
                            wd_tiles[t][:, k],
                            start=False, stop=(k == KC - 1),
                        )
                    _, slot = pair_slot(t, j)
                    nc.scalar.copy(slot, ps)
                    store_out(t, j)

            # ---- post-prologue: interleave phase-B with steady tokens so
            # the DVE combine load stays below the PE pace
            sched = []
            steady_iter = list(range(PRE, T))
            for i, t in enumerate(range(PRE)):
                sched.append(("B", t))
                if i < len(steady_iter):
                    sched.append(("s", steady_iter[i]))
            loads_order = [t for _, t in sched]

            li = 0  # rolling load emission: wd for B tokens, ws+wd for steady
            def emit_loads_until(n):
                nonlocal li
                while li < n:
                    kind, t = sched[li]
                    if kind == "B":
                        if t >= 5:
                            load_wd(t)
                    else:
                        load_ws(t)
                        load_wd(t)
                    li += 1

            emit_loads_until(4)
            for i, (kind, t) in enumerate(sched):
                emit_loads_until(min(i + 4, len(sched)))
                if kind == "B":
                    emit_phase_b(t)
                else:
                    emit_steady(t)
    nc.compile()
    return nc


_NC_CACHE = {}


def _get_nc(mode="bf16"):
    if "nc" not in _NC_CACHE:
        _NC_CACHE["nc"] = build()
    return _NC_CACHE["nc"]


MODE = "bf16"


def kernel(x, W_seasonal, b_seasonal, W_trend, b_trend, _trace=False):
    npdt = ml_dtypes.bfloat16
    nc = _get_nc()

    def to_tpkd(w):  # [T, D, C] -> [T, 128, KC, D] (c-major on partitions)
        wt = w.transpose(0, 2, 1).reshape(T, KC, 128, D)
        return np.ascontiguousarray(wt.transpose(0, 2, 1, 3))

    wst = to_tpkd(W_seasonal).astype(npdt)
    wdt = to_tpkd((W_trend - W_seasonal) / 37.0).astype(npdt)
    bias = (b_seasonal + b_trend).astype(np.float32)  # host epilogue

    in_maps = []
    for i in range(NCORES):
        xs = x[i * BC:(i + 1) * BC]                    # [BC, T, C]
        xti = xs.transpose(1, 2, 0).reshape(T, KC, 128, BC)
        xti = np.ascontiguousarray(xti.transpose(0, 2, 1, 3)).astype(npdt)
        in_maps.append({"xt": xti, "wst": wst, "wdt": wdt})

    res = run_bass_kernel_spmd(
        nc, in_maps, core_ids=list(range(NCORES)), trace=_trace
    )
    outp = np.concatenate([r["out"] for r in res.results], axis=0)
    outp = outp.astype(np.float32)
    outp += bias[None]
    if _trace:
        return outp, res
    return outp


if __name__ == "__main__":
    rng = np.random.default_rng(0)
    x = rng.standard_normal((B, T, C), dtype=np.float32)
    Ws = rng.uniform(-0.04, 0.04, (T, D, C)).astype(np.float32)
    Wt = rng.uniform(-0.04, 0.04, (T, D, C)).astype(np.float32)
    bs = rng.uniform(-0.04, 0.04, (T, D)).astype(np.float32)
    bt = rng.uniform(-0.04, 0.04, (T, D)).astype(np.float32)
    o = kernel(x, Ws, bs, Wt, bt)
    print("out shape:", o.shape, o.dtype)


# revision 18
# speedup vs baseline: 1.0840x; 1.0477x over previous
"""DLinear Trainium2 kernel (nn_DLinear_45990509805636).

Math: with T=17 and KERNEL_SIZE=37 (PAD=18), every moving-average window
covers the whole sequence plus replicated edges, so

    trend[b,t,:] = (S + (18-t)*x0 + (t+2)*x16) / 37,   S = sum_t x[:,t,:]
    out = seasonal @ Ws[t] + trend @ Wt[t] + (bs+bt)[t]
        = x_t @ Ws[t] + trend_raw_t @ Wd[t] + bias[t],
    Wd = (Wt - Ws)/37 (host-folded), trend_raw_t = P + t*Q,
    P = S + 18*x0 + 2*x16, Q = x16 - x0.

Device per core (batch shard of 512 rows), all bf16 matmuls, PE-bound at
544 N=512 matmuls (~124us); the schedule aims to keep the PE gapless:
  - x.T resident in SBUF as [c%128, t, c//128, b]; S chained on DVE behind
    the 2-token x transfers; the post-x tail (S+=x16, P) runs per-128-column
    chunk so trend unblocks ~1.5us after the last x byte lands; trend(0)
    aliases P (no copy), Q overwrites dead S.
  - prologue (t < PRE=12): phase-split. x@Ws groups (4 MMs) fill the PE
    while x/ws stream in (ws interleaved just-in-time); ACT drains psum
    straight into the f16 output pair-tile; the trend@Wd group later joins
    IN PLACE via one DVE STT (no separate outa buffers).
  - post-prologue: phase-B tokens interleaved with steady single-group
    tokens (8 MMs, ACT drain) so the DVE combine load stays under the PE
    pace; paired 2-token stores dispatched from ACT.
  - bias is NOT added on device: the host epilogue adds (bs+bt) during the
    f16->f32 upcast (saves 68 K=1 broadcast matmuls ~27us of PE time).
  - a burst of zero matmuls at t=0 warms the PE HAM clock-gate during the
    initial DMA wait so the first real matmuls run at 2.4GHz.

Sharding: data-parallel over batch, 8 cores x 512 rows; weights replicated.
"""

import sys

sys.path.insert(0, "/opt/trn_rl_repo")

import numpy as np
import ml_dtypes

from concourse import bacc
import concourse.mybir as mybir
import concourse.tile as tile
from concourse.bass_utils import run_bass_kernel_spmd

dt = mybir.dt

B, T, C, D = 4096, 17, 512, 512
NCORES = 8
BC = B // NCORES          # 512 batch rows per core
KC = C // 128             # 4 contraction chunks
JB = BC // 128            # 4 output-row tiles per core

PRE = 12                  # tokens handled phase-split (must be even)
WARM = 10                 # zero-matmuls to warm the HAM clock gate


def build():
    idt = dt.bfloat16
    nc = bacc.Bacc(None, target_bir_lowering=False, name="dlinear_v4")
    xt = nc.dram_tensor("xt", [T, 128, KC, BC], idt, kind="ExternalInput")
    wst = nc.dram_tensor("wst", [T, 128, KC, D], idt, kind="ExternalInput")
    wdt = nc.dram_tensor("wdt", [T, 128, KC, D], idt, kind="ExternalInput")
    out = nc.dram_tensor("out", [BC, T, D], dt.float16, kind="ExternalOutput")

    with tile.TileContext(nc) as tc:
        with (
            tc.tile_pool(name="xres", bufs=1) as xres,
            tc.tile_pool(name="stats", bufs=1) as stats,
            tc.tile_pool(name="wsbuf", bufs=10) as wsbuf,
            tc.tile_pool(name="wdbuf", bufs=7) as wdbuf,
            tc.tile_pool(name="tbuf", bufs=2) as tbuf,
            tc.tile_pool(name="obuf", bufs=52) as obuf,
            tc.tile_pool(name="psum", bufs=8, space="PSUM") as psum,
        ):
            xsb = xres.tile([128, T, KC, BC], idt)
            S = stats.tile([128, KC, BC], idt)
            P = stats.tile([128, KC, BC], idt)

            # HAM warm-up: matmuls on zeros while the first DMAs are in
            # flight (P's storage is scratch until the real P is written).
            # A second burst gates on the ws0 DMA so the PE-idle window
            # before the first real matmul stays under the ~3.4us HAM
            # re-throttle threshold.
            nc.vector.memset(P[:, 0], 0.0)
            wps = psum.tile([128, D], dt.float32, tag="ps", name="warm")
            for i in range(WARM):
                nc.tensor.matmul(wps, P[:, 0, 0:128], P[:, 0],
                                 start=(i == 0), stop=(i == WARM - 1))

            # ---- DMA schedule (SP): ws[t] just-in-time between x pairs so
            # the statically-ordered phase-A stream never head-of-line blocks
            ws_tiles = {}

            def load_ws(t, split=False):
                w = wsbuf.tile([128, KC, D], idt, tag="ws", name="ws")
                if split:
                    nc.sync.dma_start(w[:, 0:2], wst[t, :, 0:2])
                    nc.sync.dma_start(w[:, 2:4], wst[t, :, 2:4])
                else:
                    nc.sync.dma_start(w, wst[t])
                ws_tiles[t] = w

            def load_x(t0, t1):
                nc.sync.dma_start(
                    xsb[:, t0:t1],
                    xt[t0:t1].rearrange("t p k b -> p t k b"),
                )

            load_ws(0, split=True)
            load_x(0, 1)
            load_ws(1)
            load_x(1, 3)
            load_ws(2)
            load_x(3, 5)
            load_ws(3)
            load_x(5, 7)
            load_ws(4)
            load_x(7, 9)
            load_ws(5)
            load_x(9, 11)
            load_ws(6)
            load_x(11, 13)
            load_ws(7)
            load_x(13, 15)
            load_ws(8)
            load_x(15, 17)
            load_ws(9)

            # warm-up burst 2: moving operand is the (garbage-ok) ws0 tile,
            # so these run right after the ws0 DMA lands
            wps2 = psum.tile([128, D], dt.float32, tag="ps", name="warm2")
            for i in range(4):
                nc.tensor.matmul(wps2, P[:, 0, 0:128], ws_tiles[0][:, 0],
                                 start=(i == 0), stop=(i == 3))

            wd_tiles = {}

            def load_wd(t):
                w = wdbuf.tile([128, KC, D], idt, tag="wd", name="wd")
                nc.sync.dma_start(w, wdt[t])
                wd_tiles[t] = w

            # wd0/wd1 before the slot-blocked ws10/ws11 dispatches so they
            # are not head-of-line delayed on the SP sequencer
            load_wd(0)
            load_wd(1)
            for t in range(10, PRE):
                load_ws(t)
            for t in range(2, 5):
                load_wd(t)

            # ---- f16 per-token output tiles
            osb_tiles = {}

            def pair_slot(t, j):
                key = (t, j)
                if key not in osb_tiles:
                    osb_tiles[key] = obuf.tile(
                        [128, 1, D], dt.float16, tag="osb", name="osb")
                return osb_tiles[key], osb_tiles[key][:, 0]

            def store_out(t, j):
                nc.scalar.dma_start(
                    out[j * 128:(j + 1) * 128, t:t + 1, :],
                    osb_tiles.pop((t, j)))

            # ---- phase-A prologue: x@Ws -> psum -> ACT drain to f16 slot
            for t in range(PRE):
                for j in range(JB):
                    psa = psum.tile([128, D], dt.float32, tag="ps", name="psa")
                    for k in range(KC):
                        nc.tensor.matmul(
                            psa, xsb[:, t, k, j * 128:(j + 1) * 128],
                            ws_tiles[t][:, k],
                            start=(k == 0), stop=(k == KC - 1),
                        )
                    _, slot = pair_slot(t, j)
                    nc.scalar.copy(slot, psa)

            # ---- S chained full-width behind x arrivals; tail per-chunk
            nc.vector.tensor_tensor(S[:], xsb[:, 0], xsb[:, 1], mybir.AluOpType.add)
            for t in range(2, 16):
                nc.vector.tensor_tensor(S[:], S[:], xsb[:, t], mybir.AluOpType.add)
            for k in range(KC):
                nc.vector.tensor_tensor(S[:, k], S[:, k], xsb[:, 16, k],
                                        mybir.AluOpType.add)
                nc.vector.scalar_tensor_tensor(
                    P[:, k], xsb[:, 0, k], 18.0, S[:, k],
                    mybir.AluOpType.mult, mybir.AluOpType.add)
                nc.vector.scalar_tensor_tensor(
                    P[:, k], xsb[:, 16, k], 2.0, P[:, k],
                    mybir.AluOpType.mult, mybir.AluOpType.add)
            # Q overwrites dead S; per-chunk so trend(1) can chase phase-B(0)
            Q = S
            for k in range(KC):
                nc.vector.tensor_tensor(Q[:, k], xsb[:, 16, k], xsb[:, 0, k],
                                        mybir.AluOpType.subtract)

            def make_trend(t):
                if t == 0:
                    return P
                trend = tbuf.tile([128, KC, BC], idt, tag="trend", name="trend")
                nc.vector.tensor_scalar_mul(trend[:], Q[:], float(t))
                nc.vector.tensor_tensor(trend[:], trend[:], P[:],
                                        mybir.AluOpType.add)
                return trend

            def emit_phase_b(t):
                # trend@Wd joins the stored x@Ws part in place (DVE STT)
                trend = make_trend(t)
                for j in range(JB):
                    psb = psum.tile([128, D], dt.float32, tag="ps", name="psb")
                    for k in range(KC):
                        nc.tensor.matmul(
                            psb, trend[:, k, j * 128:(j + 1) * 128],
                            wd_tiles[t][:, k],
                            start=(k == 0), stop=(k == KC - 1),
                        )
                    _, slot = pair_slot(t, j)
                    nc.vector.scalar_tensor_tensor(
                        slot, psb, 1.0, slot,
                        mybir.AluOpType.mult, mybir.AluOpType.add,
                    )
                    store_out(t, j)

            def emit_steady(t):
                # one 8-MM group per (t, j), ACT drains straight to f16
                trend = make_trend(t)
                for j in range(JB):
                    ps = psum.tile([128, D], dt.float32, tag="ps", name="ps")
                    for k in range(KC):
                        nc.tensor.matmul(
                            ps, xsb[:, t, k, j * 128:(j + 1) * 128],
                            ws_tiles[t][:, k],
                            start=(k == 0), stop=False,
                        )
                    for k in range(KC):
                        nc.tensor.matmul(
                            ps, trend[:, k, j * 128:(j + 1) * 128],
                            wd_tiles[t][:, k],
                            start=False, stop=(k == KC - 1),
                        )
                    _, slot = pair_slot(t, j)
                    nc.scalar.copy(slot, ps)
                    store_out(t, j)

            # ---- post-prologue: interleave steady tokens among phase-B as
            # [B, B, s] triples -- per triple the PE has 14.3us of matmuls
            # while DVE needs only ~10us (combines + trend), so the PSUM
            # drains never lag the PE
            sched = []
            steady_iter = list(range(PRE, T))
            for i in range(0, PRE, 2):
                sched.append(("B", i))
                sched.append(("B", i + 1))
                if i // 2 < len(steady_iter):
                    sched.append(("s", steady_iter[i // 2]))

            li = 0  # rolling load emission: wd for B tokens, ws+wd for steady
            def emit_loads_until(n):
                nonlocal li
                while li < n:
                    kind, t = sched[li]
                    if kind == "B":
                        if t >= 5:
                            load_wd(t)
                    else:
                        load_ws(t)
                        load_wd(t)
                    li += 1

            emit_loads_until(4)
            for i, (kind, t) in enumerate(sched):
                emit_loads_until(min(i + 4, len(sched)))
                if kind == "B":
                    emit_phase_b(t)
                else:
                    emit_steady(t)
    nc.compile()
    return nc


_NC_CACHE = {}


def _get_nc(mode="bf16"):
    if "nc" not in _NC_CACHE:
        _NC_CACHE["nc"] = build()
    return _NC_CACHE["nc"]


MODE = "bf16"


def kernel(x, W_seasonal, b_seasonal, W_trend, b_trend, _trace=False):
    npdt = ml_dtypes.bfloat16
    nc = _get_nc()

    def to_tpkd(w):  # [T, D, C] -> [T, 128, KC, D] (c-major on partitions)
        wt = w.transpose(0, 2, 1).reshape(T, KC, 128, D)
        return np.ascontiguousarray(wt.transpose(0, 2, 1, 3))

    wst = to_tpkd(W_seasonal).astype(npdt)
    wdt = to_tpkd((W_trend - W_seasonal) / 37.0).astype(npdt)
    bias = (b_seasonal + b_trend).astype(np.float32)  # host epilogue

    in_maps = []
    for i in range(NCORES):
        xs = x[i * BC:(i + 1) * BC]                    # [BC, T, C]
        xti = xs.transpose(1, 2, 0).reshape(T, KC, 128, BC)
        xti = np.ascontiguousarray(xti.transpose(0, 2, 1, 3)).astype(npdt)
        in_maps.append({"xt": xti, "wst": wst, "wdt": wdt})

    res = run_bass_kernel_spmd(
        nc, in_maps, core_ids=list(range(NCORES)), trace=_trace
    )
    outp = np.concatenate([r["out"] for r in res.results], axis=0)
    outp = outp.astype(np.float32)
    outp += bias[None]
    if _trace:
        return outp, res
    return outp


if __name__ == "__main__":
    rng = np.random.default_rng(0)
    x = rng.standard_normal((B, T, C), dtype=np.float32)
    Ws = rng.uniform(-0.04, 0.04, (T, D, C)).astype(np.float32)
    Wt = rng.uniform(-0.04, 0.04, (T, D, C)).astype(np.float32)
    bs = rng.uniform(-0.04, 0.04, (T, D)).astype(np.float32)
    bt = rng.uniform(-0.04, 0.04, (T, D)).astype(np.float32)
    o = kernel(x, Ws, bs, Wt, bt)
    print("out shape:", o.shape, o.dtype)


# revision 22
# speedup vs baseline: 1.1138x; 1.0275x over previous
"""DLinear Trainium2 kernel (nn_DLinear_45990509805636).

Math: with T=17 and KERNEL_SIZE=37 (PAD=18), every moving-average window
covers the whole sequence plus replicated edges, so

    trend[b,t,:] = (S + (18-t)*x0 + (t+2)*x16) / 37,   S = sum_t x[:,t,:]
    out = seasonal @ Ws[t] + trend @ Wt[t] + (bs+bt)[t]
        = x_t @ Ws[t] + trend_raw_t @ Wd[t] + bias[t],
    Wd = (Wt - Ws)/37 (host-folded), trend_raw_t = P + t*Q,
    P = S + 18*x0 + 2*x16, Q = x16 - x0.

Device per core (batch shard of 512 rows), all bf16 matmuls, PE-bound at
544 N=512 matmuls (~124us); the schedule aims to keep the PE gapless:
  - x.T resident in SBUF as [c%128, t, c//128, b]; S chained on DVE behind
    the 2-token x transfers; the post-x tail (S+=x16, P) runs per-128-column
    chunk so trend unblocks ~1.5us after the last x byte lands; trend(0)
    aliases P (no copy), Q overwrites dead S.
  - prologue (t < PRE=12): phase-split. x@Ws groups (4 MMs) fill the PE
    while x/ws stream in (ws interleaved just-in-time); ACT drains psum
    straight into the f16 output pair-tile; the trend@Wd group later joins
    IN PLACE via one DVE STT (no separate outa buffers).
  - post-prologue: phase-B tokens interleaved with steady single-group
    tokens (8 MMs, ACT drain) so the DVE combine load stays under the PE
    pace; paired 2-token stores dispatched from ACT.
  - bias is NOT added on device: the host epilogue adds (bs+bt) during the
    f16->f32 upcast (saves 68 K=1 broadcast matmuls ~27us of PE time).
  - a burst of zero matmuls at t=0 warms the PE HAM clock-gate during the
    initial DMA wait so the first real matmuls run at 2.4GHz.

Sharding: data-parallel over batch, 8 cores x 512 rows; weights replicated.
"""

import sys

sys.path.insert(0, "/opt/trn_rl_repo")

import numpy as np
import ml_dtypes

from concourse import bacc
import concourse.mybir as mybir
import concourse.tile as tile
from concourse.bass_utils import run_bass_kernel_spmd

dt = mybir.dt

B, T, C, D = 4096, 17, 512, 512
NCORES = 8
BC = B // NCORES          # 512 batch rows per core
KC = C // 128             # 4 contraction chunks
JB = BC // 128            # 4 output-row tiles per core

PRE = 12                  # tokens handled phase-split (must be even)
WARM = 4                  # garbage-matmuls to warm the HAM clock gate


def build():
    idt = dt.bfloat16
    nc = bacc.Bacc(None, target_bir_lowering=False, name="dlinear_v4")
    xt = nc.dram_tensor("xt", [T, 128, KC, BC], idt, kind="ExternalInput")
    wst = nc.dram_tensor("wst", [T, 128, KC, D], idt, kind="ExternalInput")
    wdt = nc.dram_tensor("wdt", [T, 128, KC, D], idt, kind="ExternalInput")
    out = nc.dram_tensor("out", [BC, T, D], dt.float16, kind="ExternalOutput")

    with tile.TileContext(nc) as tc:
        with (
            tc.tile_pool(name="xres", bufs=1) as xres,
            tc.tile_pool(name="stats", bufs=1) as stats,
            tc.tile_pool(name="wsbuf", bufs=10) as wsbuf,
            tc.tile_pool(name="wdbuf", bufs=7) as wdbuf,
            tc.tile_pool(name="tbuf", bufs=2) as tbuf,
            tc.tile_pool(name="obuf", bufs=52) as obuf,
            tc.tile_pool(name="psum", bufs=8, space="PSUM") as psum,
        ):
            xsb = xres.tile([128, T, KC, BC], idt)
            S = stats.tile([128, KC, BC], idt)
            P = stats.tile([128, KC, BC], idt)

            # scratch zeros for the HAM warm-up (P's storage is scratch
            # until the real P is written)
            nc.vector.memset(P[:, 0], 0.0)

            # ---- DMA schedule (SP): ws[t] just-in-time between x pairs so
            # the statically-ordered phase-A stream never head-of-line blocks
            ws_tiles = {}

            def load_ws(t, split=False):
                w = wsbuf.tile([128, KC, D], idt, tag="ws", name="ws")
                if split:
                    nc.sync.dma_start(w[:, 0:2], wst[t, :, 0:2])
                    nc.sync.dma_start(w[:, 2:4], wst[t, :, 2:4])
                else:
                    nc.sync.dma_start(w, wst[t])
                ws_tiles[t] = w

            def load_x(t0, t1):
                nc.sync.dma_start(
                    xsb[:, t0:t1],
                    xt[t0:t1].rearrange("t p k b -> p t k b"),
                )

            load_ws(0, split=True)
            load_x(0, 1)
            load_x(1, 3)
            load_ws(1)
            load_x(3, 5)
            load_ws(2)
            load_ws(3)
            load_x(5, 7)
            load_ws(4)
            load_x(7, 9)
            load_ws(5)
            load_x(9, 11)
            load_ws(6)
            load_x(11, 13)
            load_ws(7)
            load_x(13, 15)
            load_ws(8)
            load_x(15, 17)
            load_ws(9)

            # HAM warm-up: matmuls whose moving operand is the (garbage-ok)
            # ws0 tile, so they run right after the ws0 DMA lands -- the PE
            # goes busy just before the first real matmul and stays at
            # 2.4GHz (a too-early burst would re-throttle during the
            # remaining DMA wait)
            wps2 = psum.tile([128, D], dt.float32, tag="ps", name="warm2")
            for i in range(WARM):
                nc.tensor.matmul(wps2, P[:, 0, 0:128], ws_tiles[0][:, 0],
                                 start=(i == 0), stop=(i == WARM - 1))

            wd_tiles = {}

            def load_wd(t):
                w = wdbuf.tile([128, KC, D], idt, tag="wd", name="wd")
                nc.sync.dma_start(w, wdt[t])
                wd_tiles[t] = w

            # wd0/wd1 before the slot-blocked ws10/ws11 dispatches so they
            # are not head-of-line delayed on the SP sequencer
            load_wd(0)
            load_wd(1)
            for t in range(10, PRE):
                load_ws(t)
            for t in range(2, 5):
                load_wd(t)

            # ---- f16 per-token output tiles
            osb_tiles = {}

            def pair_slot(t, j):
                key = (t, j)
                if key not in osb_tiles:
                    osb_tiles[key] = obuf.tile(
                        [128, 1, D], dt.float16, tag="osb", name="osb")
                return osb_tiles[key], osb_tiles[key][:, 0]

            def store_out(t, j):
                nc.scalar.dma_start(
                    out[j * 128:(j + 1) * 128, t:t + 1, :],
                    osb_tiles.pop((t, j)))

            # ---- phase-A prologue: x@Ws -> psum -> ACT drain to f16 slot
            for t in range(PRE):
                for j in range(JB):
                    psa = psum.tile([128, D], dt.float32, tag="ps", name="psa")
                    for k in range(KC):
                        nc.tensor.matmul(
                            psa, xsb[:, t, k, j * 128:(j + 1) * 128],
                            ws_tiles[t][:, k],
                            start=(k == 0), stop=(k == KC - 1),
                        )
                    _, slot = pair_slot(t, j)
                    nc.scalar.copy(slot, psa)

            # ---- S chained full-width behind x arrivals; tail per-chunk
            nc.vector.tensor_tensor(S[:], xsb[:, 0], xsb[:, 1], mybir.AluOpType.add)
            for t in range(2, 16):
                nc.vector.tensor_tensor(S[:], S[:], xsb[:, t], mybir.AluOpType.add)
            for k in range(KC):
                nc.vector.tensor_tensor(S[:, k], S[:, k], xsb[:, 16, k],
                                        mybir.AluOpType.add)
                nc.vector.scalar_tensor_tensor(
                    P[:, k], xsb[:, 0, k], 18.0, S[:, k],
                    mybir.AluOpType.mult, mybir.AluOpType.add)
                nc.vector.scalar_tensor_tensor(
                    P[:, k], xsb[:, 16, k], 2.0, P[:, k],
                    mybir.AluOpType.mult, mybir.AluOpType.add)
            # Q overwrites dead S; per-chunk so trend(1) can chase phase-B(0)
            Q = S
            for k in range(KC):
                nc.vector.tensor_tensor(Q[:, k], xsb[:, 16, k], xsb[:, 0, k],
                                        mybir.AluOpType.subtract)

            def make_trend(t):
                if t == 0:
                    return P
                trend = tbuf.tile([128, KC, BC], idt, tag="trend", name="trend")
                nc.vector.tensor_scalar_mul(trend[:], Q[:], float(t))
                nc.vector.tensor_tensor(trend[:], trend[:], P[:],
                                        mybir.AluOpType.add)
                return trend

            def emit_phase_b(t):
                # trend@Wd joins the stored x@Ws part in place (DVE STT)
                trend = make_trend(t)
                for j in range(JB):
                    psb = psum.tile([128, D], dt.float32, tag="ps", name="psb")
                    for k in range(KC):
                        nc.tensor.matmul(
                            psb, trend[:, k, j * 128:(j + 1) * 128],
                            wd_tiles[t][:, k],
                            start=(k == 0), stop=(k == KC - 1),
                        )
                    _, slot = pair_slot(t, j)
                    nc.vector.scalar_tensor_tensor(
                        slot, psb, 1.0, slot,
                        mybir.AluOpType.mult, mybir.AluOpType.add,
                    )
                    store_out(t, j)

            def emit_steady(t):
                # one 8-MM group per (t, j), ACT drains straight to f16
                trend = make_trend(t)
                for j in range(JB):
                    ps = psum.tile([128, D], dt.float32, tag="ps", name="ps")
                    for k in range(KC):
                        nc.tensor.matmul(
                            ps, xsb[:, t, k, j * 128:(j + 1) * 128],
                            ws_tiles[t][:, k],
                            start=(k == 0), stop=False,
                        )
                    for k in range(KC):
                        nc.tensor.matmul(
                            ps, trend[:, k, j * 128:(j + 1) * 128],
                            wd_tiles[t][:, k],
                            start=False, stop=(k == KC - 1),
                        )
                    _, slot = pair_slot(t, j)
                    nc.scalar.copy(slot, ps)
                    store_out(t, j)

            # ---- post-prologue: interleave steady tokens among phase-B as
            # [B, B, s] triples -- per triple the PE has 14.3us of matmuls
            # while DVE needs only ~10us (combines + trend), so the PSUM
            # drains never lag the PE
            sched = []
            steady_iter = list(range(PRE, T))
            for i in range(0, PRE, 2):
                sched.append(("B", i))
                sched.append(("B", i + 1))
                if i // 2 < len(steady_iter):
                    sched.append(("s", steady_iter[i // 2]))

            li = 0  # rolling load emission: wd for B tokens, ws+wd for steady
            def emit_loads_until(n):
                nonlocal li
                while li < n:
                    kind, t = sched[li]
                    if kind == "B":
                        if t >= 5:
                            load_wd(t)
                    else:
                        load_ws(t)
                        load_wd(t)
                    li += 1

            emit_loads_until(4)
            for i, (kind, t) in enumerate(sched):
                emit_loads_until(min(i + 4, len(sched)))
                if kind == "B":
                    emit_phase_b(t)
                else:
                    emit_steady(t)
    nc.compile()
    return nc


_NC_CACHE = {}


def _get_nc(mode="bf16"):
    if "nc" not in _NC_CACHE:
        _NC_CACHE["nc"] = build()
    return _NC_CACHE["nc"]


MODE = "bf16"


def kernel(x, W_seasonal, b_seasonal, W_trend, b_trend, _trace=False):
    npdt = ml_dtypes.bfloat16
    nc = _get_nc()

    def to_tpkd(w):  # [T, D, C] -> [T, 128, KC, D] (c-major on partitions)
        wt = w.transpose(0, 2, 1).reshape(T, KC, 128, D)
        return np.ascontiguousarray(wt.transpose(0, 2, 1, 3))

    wst = to_tpkd(W_seasonal).astype(npdt)
    wdt = to_tpkd((W_trend - W_seasonal) / 37.0).astype(npdt)
    bias = (b_seasonal + b_trend).astype(np.float32)  # host epilogue

    in_maps = []
    for i in range(NCORES):
        xs = x[i * BC:(i + 1) * BC]                    # [BC, T, C]
        xti = xs.transpose(1, 2, 0).reshape(T, KC, 128, BC)
        xti = np.ascontiguousarray(xti.transpose(0, 2, 1, 3)).astype(npdt)
        in_maps.append({"xt": xti, "wst": wst, "wdt": wdt})

    res = run_bass_kernel_spmd(
        nc, in_maps, core_ids=list(range(NCORES)), trace=_trace
    )
    outp = np.concatenate([r["out"] for r in res.results], axis=0)
    outp = outp.astype(np.float32)
    outp += bias[None]
    if _trace:
        return outp, res
    return outp


if __name__ == "__main__":
    rng = np.random.default_rng(0)
    x = rng.standard_normal((B, T, C), dtype=np.float32)
    Ws = rng.uniform(-0.04, 0.04, (T, D, C)).astype(np.float32)
    Wt = rng.uniform(-0.04, 0.04, (T, D, C)).astype(np.float32)
    bs = rng.uniform(-0.04, 0.04, (T, D)).astype(np.float32)
    bt = rng.uniform(-0.04, 0.04, (T, D)).astype(np.float32)
    o = kernel(x, Ws, bs, Wt, bt)
    print("out shape:", o.shape, o.dtype)
